# revision 10
# baseline (speedup 1.0000x reference)
"""Trainium2 Bass kernel for CSMultiHeadAttention (rotated cross-chunk MHA).

Sharding: data-parallel over batch (B=8) across the 8 NeuronCores; each core
computes one batch element end-to-end (no collectives).

Per-core dataflow (all matmuls bf16 inputs, fp32 PSUM accumulation):
  prep: x,W -> cast bf16 -> DRAM scratch -> DMA-transpose -> x^T, W^T in SBUF
  proj: Q^T = (Wq^T)^T stationary @ x^T moving (+bq), K^T likewise,
        V    = (x^T)^T stationary @ Wv^T moving (+bv) into head-strided
        V_aug layout [n, h, 65] with a ones column per head (gives softmax
        denominators for free during the AV matmul).
  attn (transposed layout, head-pair row-tiling on the 128x128 PE array):
        energy^T[k,q] = K^T_h.T @ Q^T_h  (d=64 contraction; heads 2j/2j+1
        occupy partition halves 0-63/64-127 -> concurrent row-tiled matmuls)
        att^T = exp(energy^T * 1/sqrt(E))  on ScalarE, PSUM->SBUF bf16
        out_aug^T[65,q] = V_aug.T @ att^T  (row 64 = softmax denominator)
        normalize: recip(denominator row) -> DMA partition-broadcast ->
        attout^T = out^T * recip  (bf16, already in the [e,n] layout the
        output projection needs as its stationary operand)
  proj2: y[n,f] = (attout^T).T stationary @ Wp^T moving + bp -> DRAM fp32
"""

import numpy as np

import concourse.bass as bass
import concourse.tile as tile
from concourse import bacc
from concourse import mybir
from concourse import bass_utils

F32 = mybir.dt.float32
BF16 = mybir.dt.bfloat16

B, S, E, H = 8, 3072, 512, 8
C = 3                # seq chunks
N = S // C           # 1024 tokens per chunk
D = E // H           # 64 head dim
P = 128              # partitions
ET = E // P          # 4 feature tiles
NT = N // P          # 8 token tiles per chunk
FREE = 512           # matmul moving free dim / PSUM bank (fp32)
NQ = N // FREE       # 2 q-halves per chunk
SCALE = float(1.0 / np.sqrt(np.float32(E)))
QSEL = [1, 2, 0]     # out chunk c uses Q of chunk QSEL[c]
KSEL = [2, 0, 1]     # ... and K,V of chunk KSEL[c]

_CACHE = {}


def _bcast_part(ap, nparts):
    """View a single-partition AP broadcast across nparts partitions."""
    return bass.AP(tensor=ap.tensor, offset=ap.offset,
                   ap=[[0, nparts]] + list(ap.ap)[1:])


def build_bass():
    nc = bacc.Bacc()
    x = nc.dram_tensor("x", [S, E], F32, kind="ExternalInput")
    W = {nm: nc.dram_tensor(nm, [C, E, E], F32, kind="ExternalInput")
         for nm in ("Wq", "Wk", "Wv", "Wp")}
    bias = {nm: nc.dram_tensor(nm, [C, E], F32, kind="ExternalInput")
            for nm in ("bq", "bk", "bv", "bp")}
    out = nc.dram_tensor("out", [S, E], F32, kind="ExternalOutput")

    with tile.TileContext(nc) as tc:
        with (
            tc.tile_pool(name="dram", bufs=1, space="DRAM") as dram,
            tc.tile_pool(name="persist", bufs=1) as persist,
            tc.tile_pool(name="mm_ps", bufs=2, space="PSUM") as mm_ps,
            tc.tile_pool(name="en_ps", bufs=3, space="PSUM") as en_ps,
            tc.tile_pool(name="av_ps", bufs=3, space="PSUM") as av_ps,
        ):
            # ---- bias tiles ----
            # bq/bk in per-partition layout [128, ET] (col j = feature tile j)
            bqT, bkT, bv_bc, bp_bc = [], [], [], []
            for c in range(C):
                t_bq = persist.tile([P, ET], F32, name=f"bqT_{c}")
                nc.sync.dma_start(out=t_bq, in_=bias["bq"][c].rearrange(
                    "(j p) -> p j", p=P))
                bqT.append(t_bq)
                t_bk = persist.tile([P, ET], F32, name=f"bkT_{c}")
                nc.sync.dma_start(out=t_bk, in_=bias["bk"][c].rearrange(
                    "(j p) -> p j", p=P))
                bkT.append(t_bk)
                # bv/bp broadcast along partitions [128, 512]
                t_bv = persist.tile([P, E], F32, name=f"bv_bc_{c}")
                nc.sync.dma_start(out=t_bv, in_=_bcast_part(bias["bv"][c:c + 1], P))
                bv_bc.append(t_bv)
                t_bp = persist.tile([P, E], F32, name=f"bp_bc_{c}")
                nc.sync.dma_start(out=t_bp, in_=_bcast_part(bias["bp"][c:c + 1], P))
                bp_bc.append(t_bp)

            # W_p^T persists until phase D
            WpT = [[persist.tile([P, E], BF16, name=f"WpT_{c}_{k}")
                    for k in range(ET)] for c in range(C)]
            # persistent projection outputs
            QT = [[persist.tile([P, N], BF16, name=f"QT_{c}_{j}")
                   for j in range(ET)] for c in range(C)]
            KT = [[persist.tile([P, N], BF16, name=f"KT_{c}_{j}")
                   for j in range(ET)] for c in range(C)]
            # V_aug: [n-tile][128, H, D+1]; per head 64 V columns + ones col
            Vaug = [[persist.tile([P, H, D + 1], BF16, name=f"Vaug_{c}_{i}")
                     for i in range(NT)] for c in range(C)]

            # scoped pools for prep + x^T and W^{q,k,v}T (released after B)
            with (
                tc.tile_pool(name="io", bufs=4) as io,
                tc.tile_pool(name="xT", bufs=1) as xpool,
                tc.tile_pool(name="wT", bufs=1) as wpool,
            ):
                # ---- phase A: cast x and W to bf16, round-trip through
                # DRAM with DMA-transpose to get x^T / W^T in SBUF ----
                x_bf = dram.tile([S, E], BF16)
                for t in range(S // P):
                    ld = io.tile([P, E], F32, tag="ld", name=f"ldx_{t}")
                    nc.sync.dma_start(out=ld, in_=x[t * P:(t + 1) * P, :])
                    cs = io.tile([P, E], BF16, tag="cs", name=f"csx_{t}")
                    nc.vector.tensor_copy(out=cs, in_=ld)
                    nc.sync.dma_start(out=x_bf[t * P:(t + 1) * P, :], in_=cs)

                W_bf = {}
                for nm in ("Wq", "Wk", "Wv", "Wp"):
                    wb = dram.tile([C, E, E], BF16, name=f"{nm}_bf")
                    W_bf[nm] = wb
                    for c in range(C):
                        for t in range(ET):
                            ld = io.tile([P, E], F32, tag="ld",
                                         name=f"ld_{nm}_{c}_{t}")
                            nc.sync.dma_start(
                                out=ld, in_=W[nm][c, t * P:(t + 1) * P, :])
                            cs = io.tile([P, E], BF16, tag="cs",
                                         name=f"cs_{nm}_{c}_{t}")
                            nc.vector.tensor_copy(out=cs, in_=ld)
                            nc.sync.dma_start(
                                out=wb[c, t * P:(t + 1) * P, :], in_=cs)

                for c in range(C):
                    for k in range(ET):
                        nc.sync.dma_start(
                            out=WpT[c][k],
                            in_=W_bf["Wp"][c, :, k * P:(k + 1) * P],
                            transpose=True)
                xT = [[xpool.tile([P, N], BF16, name=f"xT_{c}_{k}")
                       for k in range(ET)] for c in range(C)]
                for c in range(C):
                    for k in range(ET):
                        nc.sync.dma_start(
                            out=xT[c][k],
                            in_=x_bf[c * N:(c + 1) * N, k * P:(k + 1) * P],
                            transpose=True)
                WT = {}
                for nm in ("Wq", "Wk", "Wv"):
                    WT[nm] = [[wpool.tile([P, E], BF16, name=f"{nm}T_{c}_{k}")
                               for k in range(ET)] for c in range(C)]
                    for c in range(C):
                        for k in range(ET):
                            nc.sync.dma_start(
                                out=WT[nm][c][k],
                                in_=W_bf[nm][c, :, k * P:(k + 1) * P],
                                transpose=True)

                # ---- phase B: Q^T / K^T / V projections ----
                for c in range(C):
                    # Q^T, K^T: [e_out tile j][128, N]
                    for j in range(ET):
                        for qh in range(NQ):
                            for wt, bt, dst in ((WT["Wq"], bqT, QT),
                                                (WT["Wk"], bkT, KT)):
                                ps = mm_ps.tile([P, FREE], F32, tag="mm",
                                                name=f"ps_qk_{c}_{j}_{qh}")
                                for k in range(ET):
                                    nc.tensor.matmul(
                                        ps,
                                        lhsT=wt[c][k][:, j * P:(j + 1) * P],
                                        rhs=xT[c][k][:, qh * FREE:(qh + 1) * FREE],
                                        start=(k == 0), stop=(k == ET - 1))
                                nc.vector.tensor_scalar_add(
                                    dst[c][j][:, qh * FREE:(qh + 1) * FREE],
                                    ps, bt[c][:, j:j + 1])
                    # V: [n-tile i][128, H, D+1]
                    for i in range(NT):
                        ps = mm_ps.tile([P, FREE], F32, tag="mm",
                                        name=f"ps_v_{c}_{i}")
                        for k in range(ET):
                            nc.tensor.matmul(
                                ps,
                                lhsT=xT[c][k][:, i * P:(i + 1) * P],
                                rhs=WT["Wv"][c][k],
                                start=(k == 0), stop=(k == ET - 1))
                        nc.vector.tensor_add(
                            out=Vaug[c][i][:, :, 0:D],
                            in0=ps.rearrange("p (h d) -> p h d", d=D),
                            in1=bv_bc[c].rearrange("p (h d) -> p h d", d=D))
                        nc.vector.memset(Vaug[c][i][:, :, D:D + 1], 1.0)

            # ---- phase C: attention, head pairs (2j, 2j+1) ----
            attn_pools = (
                tc.tile_pool(name="ao", bufs=1),
                tc.tile_pool(name="attw", bufs=6),
                tc.tile_pool(name="norm", bufs=4),
                tc.tile_pool(name="yout", bufs=4),
            )
            aop, attw, normp, yout = [p.__enter__() for p in attn_pools]
            aoT = [[None] * ET for _ in range(C)]  # attout^T [e tile][128, N]
            for c in range(C):
                qc, kc = QSEL[c], KSEL[c]
                for j in range(ET):
                    ao = aop.tile([P, N], BF16, name=f"aoT_{c}_{j}")
                    aoT[c][j] = ao
                    for qh in range(NQ):
                        av_ts = []
                        for hh in range(2):
                            av = av_ps.tile([D + 1, FREE], F32, tag="av",
                                            name=f"av_{c}_{j}_{qh}_{hh}")
                            av_ts.append(av)
                        for kt in range(NT):
                            en_ts = []
                            for hh in range(2):
                                bp0 = D * hh
                                en = en_ps.tile([P, FREE], F32, tag="en",
                                                name=f"en_{c}_{j}_{qh}_{kt}_{hh}")
                                nc.tensor.matmul(
                                    en,
                                    lhsT=KT[kc][j][bp0:bp0 + D,
                                                   kt * P:(kt + 1) * P],
                                    rhs=QT[qc][j][bp0:bp0 + D,
                                                  qh * FREE:(qh + 1) * FREE],
                                    start=True, stop=True)
                                en_ts.append(en)
                            at_ts = []
                            for hh in range(2):
                                at = attw.tile([P, FREE], BF16, tag="at",
                                               name=f"at_{c}_{j}_{qh}_{kt}_{hh}")
                                nc.scalar.activation(
                                    out=at, in_=en_ts[hh],
                                    func=mybir.ActivationFunctionType.Exp,
                                    scale=SCALE)
                                at_ts.append(at)
                            for hh in range(2):
                                h = 2 * j + hh
                                nc.tensor.matmul(
                                    av_ts[hh],
                                    lhsT=Vaug[kc][kt][:, h, :],
                                    rhs=at_ts[hh],
                                    start=(kt == 0), stop=(kt == NT - 1))
                        for hh in range(2):
                            recip = normp.tile([1, FREE], F32, tag="recip",
                                               name=f"rc_{c}_{j}_{qh}_{hh}")
                            nc.vector.reciprocal(recip, av_ts[hh][D:D + 1, :])
                            # partition-broadcast via DRAM bounce (SBUF
                            # source DMAs can't have a zero partition step)
                            rd = dram.tile([1, FREE], F32,
                                           name=f"rd_{c}_{j}_{qh}_{hh}")
                            nc.sync.dma_start(out=rd, in_=recip)
                            rb = normp.tile([D, FREE], F32, tag="rb",
                                            name=f"rb_{c}_{j}_{qh}_{hh}")
                            nc.sync.dma_start(out=rb, in_=_bcast_part(rd, D))
                            nc.vector.tensor_mul(
                                ao[D * hh:D * hh + D,
                                   qh * FREE:(qh + 1) * FREE],
                                av_ts[hh][0:D, :], rb)

            # ---- phase D: output projection ----
            for c in range(C):
                for i in range(NT):
                    ps = mm_ps.tile([P, FREE], F32, tag="mm",
                                    name=f"ps_y_{c}_{i}")
                    for k in range(ET):
                        nc.tensor.matmul(
                            ps,
                            lhsT=aoT[c][k][:, i * P:(i + 1) * P],
                            rhs=WpT[c][k],
                            start=(k == 0), stop=(k == ET - 1))
                    y = yout.tile([P, E], F32, tag="y", name=f"y_{c}_{i}")
                    nc.vector.tensor_add(out=y, in0=ps, in1=bp_bc[c])
                    nc.sync.dma_start(
                        out=out[c * N + i * P:c * N + (i + 1) * P, :], in_=y)

            for p in reversed(attn_pools):
                p.__exit__(None, None, None)

    nc.finalize()
    return nc


def _make_runner(nc, n_cores):
    """Build a cached shard_map-jitted executor for the prebuilt Bass module
    (same lowering as bass2jax.run_bass_via_pjrt, but jitted once so repeated
    calls skip retracing/recompile)."""
    import jax
    from jax.sharding import Mesh, PartitionSpec
    from jax.experimental.shard_map import shard_map
    from concourse import mybir as _mybir
    from concourse.bass2jax import (
        _bass_exec_p, install_neuronx_cc_hook, partition_id_tensor)

    install_neuronx_cc_hook()

    partition_name = (nc.partition_id_tensor.name
                      if nc.partition_id_tensor else None)
    in_names, out_names, out_avals, zero_outs = [], [], [], []
    for alloc in nc.m.functions[0].allocations:
        if not isinstance(alloc, _mybir.MemoryLocationSet):
            continue
        name = alloc.memorylocations[0].name
        if alloc.kind == "ExternalInput":
            if name != partition_name:
                in_names.append(name)
        elif alloc.kind == "ExternalOutput":
            shape = tuple(alloc.tensor_shape)
            dtype = _mybir.dt.np(alloc.dtype)
            out_names.append(name)
            out_avals.append(jax.core.ShapedArray(shape, dtype))
            zero_outs.append(np.zeros(shape, dtype))
    n_params = len(in_names)
    all_names = in_names + out_names
    if partition_name is not None:
        all_names.append(partition_name)

    def _body(*args):
        operands = list(args)
        if partition_name is not None:
            operands.append(partition_id_tensor())
        return tuple(_bass_exec_p.bind(
            *operands,
            out_avals=tuple(out_avals),
            in_names=tuple(all_names),
            out_names=tuple(out_names),
            lowering_input_output_aliases=(),
            sim_require_finite=True,
            sim_require_nnan=True,
            nc=nc,
        ))

    devices = jax.devices()[:n_cores]
    mesh = Mesh(np.asarray(devices), ("core",))
    nin = n_params + len(out_names)
    sharded = jax.jit(
        shard_map(_body, mesh=mesh,
                  in_specs=(PartitionSpec("core"),) * nin,
                  out_specs=(PartitionSpec("core"),) * len(out_names),
                  check_rep=False),
        keep_unused=True)
    return sharded, in_names, out_names, out_avals, zero_outs


def get_runner():
    if "runner" not in _CACHE:
        if "nc" not in _CACHE:
            _CACHE["nc"] = build_bass()
        _CACHE["runner"] = _make_runner(_CACHE["nc"], B)
    return _CACHE["runner"]


def kernel(**inputs):
    sharded, in_names, out_names, out_avals, zero_outs = get_runner()

    x = np.ascontiguousarray(np.asarray(inputs["x"], dtype=np.float32))
    shared = {nm: np.ascontiguousarray(np.asarray(inputs[nm], np.float32))
              for nm in ("Wq", "bq", "Wk", "bk", "Wv", "bv", "Wp", "bp")}
    per_core = [[np.asarray(dict(shared, x=x[b])[nm]) for nm in in_names]
                for b in range(B)]
    concat_in = [np.concatenate([per_core[b][i] for b in range(B)], axis=0)
                 for i in range(len(in_names))]
    concat_zeros = [np.zeros((B * z.shape[0], *z.shape[1:]), z.dtype)
                    for z in zero_outs]
    out_arrs = sharded(*concat_in, *concat_zeros)
    o = np.asarray(out_arrs[out_names.index("out")])
    return o.reshape(B, *out_avals[out_names.index("out")].shape)
